# revision 1
# baseline (speedup 1.0000x reference)
"""Bass/Trainium2 kernel for BoundaryAwareDiceLoss (data-parallel over 8 NeuronCores).

Math (matches the jax reference):
  dice  = 1 - (2*sum(p*t) + 1e-5) / (sum(p) + sum(t) + 1e-5)
  bce   = -mean(t*log(p) + (1-t)*log(1-p))
  bmask = fg & (any of the 6 axis-neighbors (b+-1, h+-1, w+-1), edge-clamped, is background)
  out   = dice + 10 * bce * mean(bmask)

Since t is exactly {0,1}, the boundary test is integer counting:
  total7 = fg + sum(6 clamped neighbors) ; non-boundary-fg = relu(total7 - 6)
  sum(bmask) = sum(t) - sum(relu(total7 - 6))

Per-core layout (4 owned batch planes + 2 halo planes):
  target SBUF [128, K*SLOTS*WB] bf16, partition p = h within 128-row block k,
  free axis blocks (k, slot, w) with w data at cols [2, 514) and clamp pads at
  cols 1 / 514 (so +-1 shifted access patterns implement w-clamping).
  All neighbor shifts reduce to PE matmuls accumulating into one PSUM bank per
  block: tridiagonal weights (h+-1), identity over slot+-1 (b+-1) and over
  w-shifted columns (w+-1), single-entry weights for cross-k boundary rows.
Per-core partial sums ([128,12] f32) are combined on the host in float64.
"""

import numpy as np
import ml_dtypes

BF16 = ml_dtypes.bfloat16

B_TOTAL, C, H, W = 32, 1, 512, 512
NCORES = 8
B_OWN = B_TOTAL // NCORES  # 4
P = 128
K = H // P  # 4
SLOTS = B_OWN + 2  # 6 (halo_lo, b0..b3, halo_hi)
WB = 516  # padded block width: [filler, lpad, w0..w511, rpad, filler]
TBW = K * SLOTS * WB
PBW = K * B_OWN * W
NPIX = float(B_TOTAL * C * H * W)
WEIGHT = 10.0
SMOOTH = 1e-5

_CACHE = {}


def _make_weights():
    # matmul computes out[m,n] = sum_p lhsT[p,m] * rhs[p,n]
    wnp = np.zeros((6, P, P), np.float32)
    for m in range(P):
        if m > 0:
            wnp[0][m - 1, m] = 1.0  # out[m] += in[m-1]  (h-1)
        if m < P - 1:
            wnp[0][m + 1, m] = 1.0  # out[m] += in[m+1]  (h+1)
    wnp[0] += np.eye(P, dtype=np.float32)  # + fg itself (total7 = fg + 6 nbrs)
    wnp[1] = wnp[0]
    wnp[1][0, 0] += 1.0  # k==0: h-1 clamps to h=0 (self)
    wnp[2] = wnp[0]
    wnp[2][P - 1, P - 1] += 1.0  # k==K-1: h+1 clamps to h=511 (self)
    wnp[3] = np.eye(P, dtype=np.float32)
    wnp[4][P - 1, 0] = 1.0  # up-fix: out[0] += prev-k-block row 127
    wnp[5][0, P - 1] = 1.0  # dn-fix: out[127] += next-k-block row 0
    return wnp.astype(BF16)


def _build_nc(nrep=1):
    import concourse.bacc as bacc
    import concourse.mybir as mybir
    from concourse.tile import TileContext

    dt = mybir.dt
    alu = mybir.AluOpType
    act = mybir.ActivationFunctionType

    nc = bacc.Bacc("TRN2", target_bir_lowering=False)
    pred_d = nc.dram_tensor("pred", [B_OWN, K, P, W], dt.float32, kind="ExternalInput")
    tgt_d = nc.dram_tensor("target", [SLOTS, K, P, W], dt.bfloat16, kind="ExternalInput")
    out_d = nc.dram_tensor("out", [P, 12], dt.float32, kind="ExternalOutput")
    wts_d = nc.inline_tensor(_make_weights(), name="wts")

    with TileContext(nc) as tc:
        with (
            tc.tile_pool(name="big", bufs=1) as big,
            tc.tile_pool(name="rs", bufs=2) as rsp,
            tc.tile_pool(name="ps", bufs=8, space="PSUM") as psp,
        ):
            tb = big.tile([P, TBW], dt.bfloat16)
            pb = big.tile([P, PBW], dt.float32)
            logp = big.tile([P, PBW], dt.bfloat16)
            l1p = big.tile([P, PBW], dt.bfloat16)
            dd = big.tile([P, PBW], dt.bfloat16)
            # racc2 viewed [P, 10, 4], reduced over last axis -> parts10:
            #   g0-g3 boundary blocks (16), g4 sum(t)/k, g5 sum(p*t)/k,
            #   g6 sum(t*d)/k, g7 sum(p)/k, g8 sum(log1mp)/k, g9 zero
            racc2 = big.tile([P, 40], dt.float32)
            parts10 = big.tile([P, 10], dt.float32)
            fin = big.tile([P, 12], dt.float32)
            wsb = big.tile([P, 6 * P], dt.bfloat16)
            bneg6 = big.tile([P, 1], dt.float32)

            nc.vector.memset(racc2[:], 0.0)
            nc.vector.memset(bneg6[:], -6.0)

            wsbv = wsb[:].rearrange("p (i m) -> p i m", i=6)
            tbv = tb[:].rearrange("p (k s wb) -> p k s wb", k=K, s=SLOTS)
            tbk = tb[:].rearrange("p (k x wb) -> p k x wb", k=K, wb=WB)
            pbv = pb[:].rearrange("p (k b w) -> p k b w", k=K, b=B_OWN)
            logpv = logp[:].rearrange("p (k b w) -> p k b w", k=K, b=B_OWN)
            l1pv = l1p[:].rearrange("p (k b w) -> p k b w", k=K, b=B_OWN)
            ddv = dd[:].rearrange("p (k b w) -> p k b w", k=K, b=B_OWN)

            for _rep in range(nrep):
                nc.sync.dma_start(out=wsbv, in_=wts_d[:].rearrange("i p m -> p i m"))
                # DMA order: t0, t1, p0, t2, p1, t3, p2, p3 (boundary k needs
                # target chunks k-1..k+1; elementwise k needs pred chunk k)
                def dma_t(k):
                    nc.sync.dma_start(
                        out=tbv[:, k, :, 2 : 2 + W],
                        in_=tgt_d[:, k].rearrange("s p w -> p s w"),
                    )
                    nc.vector.tensor_copy(out=tbk[:, k, :, 1:2], in_=tbk[:, k, :, 2:3])
                    nc.vector.tensor_copy(
                        out=tbk[:, k, :, WB - 2 : WB - 1],
                        in_=tbk[:, k, :, WB - 3 : WB - 2],
                    )

                def dma_p(k):
                    nc.sync.dma_start(
                        out=pbv[:, k], in_=pred_d[:, k].rearrange("b p w -> p b w")
                    )

                dma_t(0)
                dma_t(1)
                dma_p(0)
                dma_t(2)
                dma_p(1)
                dma_t(3)
                dma_p(2)
                dma_p(3)

                def boundary(k):
                    # matmuls grouped by weight matrix across the 4 b-blocks
                    # (fewer PE weight switches); each block accumulates into
                    # its own PSUM bank.
                    t3i = 1 if k == 0 else (2 if k == K - 1 else 0)
                    pss = [psp.tile([P, W], dt.float32, name="pss", tag="pss") for _ in range(B_OWN)]
                    groups = [
                        (t3i, [lambda s: tbv[:, k, s, 2 : 2 + W]]),  # h+-1 (+fg)
                        (
                            3,
                            [
                                lambda s: tbv[:, k, s - 1, 2 : 2 + W],  # b-1
                                lambda s: tbv[:, k, s + 1, 2 : 2 + W],  # b+1
                                lambda s: tbv[:, k, s, 1 : 1 + W],  # w-1 (lpad)
                                lambda s: tbv[:, k, s, 3 : 3 + W],  # w+1 (rpad)
                            ],
                        ),
                    ]
                    if k > 0:
                        groups.append((4, [lambda s: tbv[:, k - 1, s, 2 : 2 + W]]))
                    if k < K - 1:
                        groups.append((5, [lambda s: tbv[:, k + 1, s, 2 : 2 + W]]))
                    n_per_block = sum(len(fns) for _, fns in groups)
                    cnt = [0] * B_OWN
                    for wi, fns in groups:
                        for fn in fns:
                            for b in range(B_OWN):
                                nc.tensor.matmul(
                                    pss[b][:],
                                    wsbv[:, wi, :],
                                    fn(b + 1),
                                    start=(cnt[b] == 0),
                                    stop=(cnt[b] == n_per_block - 1),
                                )
                                cnt[b] += 1
                    for b in range(B_OWN):
                        # non-boundary-fg count: total7 == 7. Drains alternate
                        # DVE (is_ge) / ACT (relu(x-6), exact on integers) to
                        # balance engine load. HW accum_out reduces with op1
                        # on DVE (must be add); ACT accum is always add.
                        if b % 2 == 0:
                            rs = rsp.tile([P, W], dt.bfloat16, name="rs", tag="rs")
                            nc.vector.tensor_scalar(
                                out=rs[:],
                                in0=pss[b][:],
                                scalar1=6.5,
                                scalar2=0.0,
                                op0=alu.is_ge,
                                op1=alu.add,
                                accum_out=racc2[:, 4 * k + b : 4 * k + b + 1],
                            )
                        else:
                            rs = rsp.tile([P, W], dt.bfloat16, name="rs", tag="rs")
                            nc.scalar.activation(
                                out=rs[:],
                                in_=pss[b][:],
                                func=act.Relu,
                                bias=bneg6[:, 0:1],
                                scale=1.0,
                                accum_out=racc2[:, 4 * k + b : 4 * k + b + 1],
                            )

                def elementwise(k):
                    t_own_k = tbv[:, k, 1 : 1 + B_OWN, 2 : 2 + W]  # [p, b, w]
                    # sum(pred) per k (out -> dd scratch, overwritten by d later)
                    nc.vector.tensor_scalar(
                        out=ddv[:, k], in0=pbv[:, k], scalar1=0.0, scalar2=0.0,
                        op0=alu.add, op1=alu.add,
                        accum_out=racc2[:, 28 + k : 29 + k],
                    )
                    # sum(t) per k
                    nc.vector.tensor_scalar(
                        out=ddv[:, k], in0=t_own_k, scalar1=0.0, scalar2=0.0,
                        op0=alu.add, op1=alu.add,
                        accum_out=racc2[:, 16 + k : 17 + k],
                    )
                    # logs on ScalarE; sum(log1p(-p)) via the accumulator
                    nc.scalar.activation(out=logpv[:, k], in_=pbv[:, k], func=act.Ln)
                    nc.scalar.activation(
                        out=l1pv[:, k], in_=pbv[:, k], func=act.Ln,
                        bias=1.0, scale=-1.0,
                        accum_out=racc2[:, 32 + k : 33 + k],
                    )
                    # d = log(p) - log(1-p)
                    nc.vector.tensor_sub(ddv[:, k], logpv[:, k], l1pv[:, k])
                    # p*t and t*d on GpSimd (overwrite l1p / logp as scratch)
                    nc.gpsimd.tensor_mul(l1pv[:, k], pbv[:, k], t_own_k)
                    nc.gpsimd.tensor_mul(logpv[:, k], t_own_k, ddv[:, k])
                    # sum(p*t) per k
                    nc.vector.tensor_scalar(
                        out=ddv[:, k], in0=l1pv[:, k], scalar1=0.0, scalar2=0.0,
                        op0=alu.add, op1=alu.add,
                        accum_out=racc2[:, 20 + k : 21 + k],
                    )
                    # sum(t*d) per k
                    nc.vector.tensor_scalar(
                        out=ddv[:, k], in0=logpv[:, k], scalar1=0.0, scalar2=0.0,
                        op0=alu.add, op1=alu.add,
                        accum_out=racc2[:, 24 + k : 25 + k],
                    )

                for k in range(K):
                    boundary(k)
                    elementwise(k)

                nc.vector.tensor_reduce(
                    out=parts10[:],
                    in_=racc2[:].rearrange("p (g n) -> p g n", n=4),
                    axis=mybir.AxisListType.X,
                    op=alu.add,
                )
                nc.vector.tensor_copy(out=fin[:, 0:10], in_=parts10[:])
                nc.vector.memset(fin[:, 10:12], 0.0)

                nc.sync.dma_start(out=out_d[:], in_=fin[:])

    nc.compile()
    return nc


def _get_nc(nrep=1):
    if nrep not in _CACHE:
        _CACHE[nrep] = _build_nc(nrep)
    return _CACHE[nrep]


def _shard_inputs(pred, target):
    pred = np.ascontiguousarray(np.asarray(pred, dtype=np.float32)).reshape(
        B_TOTAL, H, W
    )
    tgt = np.asarray(target, dtype=np.float32).reshape(B_TOTAL, H, W)
    tgt_bf = tgt.astype(BF16)
    in_maps = []
    for c in range(NCORES):
        b0 = c * B_OWN
        pred_c = pred[b0 : b0 + B_OWN].reshape(B_OWN, K, P, W)
        lo = max(b0 - 1, 0)
        hi = min(b0 + B_OWN, B_TOTAL - 1)
        idx = [lo] + list(range(b0, b0 + B_OWN)) + [hi]
        tgt_c = np.ascontiguousarray(tgt_bf[idx]).reshape(SLOTS, K, P, W)
        in_maps.append({"pred": pred_c, "target": tgt_c})
    return in_maps


def _combine(parts_list):
    S = np.zeros(12, dtype=np.float64)
    for r in parts_list:
        S += np.asarray(r, dtype=np.float64).sum(axis=0)
    s_r = S[0] + S[1] + S[2] + S[3]  # sum(total7==7) = non-boundary fg
    s_t = S[4]
    s_pt = S[5]
    s_e = S[6]
    s_p = S[7]
    s_l1p = S[8]
    dice = 1.0 - (2.0 * s_pt + SMOOTH) / (s_p + s_t + SMOOTH)
    bce = -(s_e + s_l1p) / NPIX
    mb = (s_t - s_r) / NPIX
    return np.asarray(dice + WEIGHT * bce * mb, dtype=np.float32)


TRACE = False
LAST_RESULTS = None


def kernel(pred, target):
    global LAST_RESULTS
    from concourse.bass_utils import run_bass_kernel_spmd

    in_maps = _shard_inputs(pred, target)
    nc = _get_nc()
    res = run_bass_kernel_spmd(
        nc, in_maps, core_ids=list(range(NCORES)), trace=TRACE
    )
    LAST_RESULTS = res
    return _combine([r["out"] for r in res.results])

